# revision 1
# baseline (speedup 1.0000x reference)
"""EquiMultiHeadAttention on 8 Trainium2 NeuronCores.

Sharding: one attention head per core (H=8, n_cores=8). Each core computes,
for all 4 batches, its head's q/k/v projections, the full SxS attention, and
that head's contribution to the output projection. The host sums the 8
partial outputs and adds the output bias (scalar blade only).

Math folded into per-head host-precomputed weights:
  - q is packed to the 8 surviving mv components of the PGA inner product,
    pre-scaled by 1/sqrt(32); k packed identically -> the score matmul is a
    plain K=128 contraction.
  - The output projection (W_out columns of this head) is applied to v
    *before* attention (it commutes with the softmax normalization), so the
    attention's second matmul directly produces this head's output
    contribution. An extra all-ones column on v yields the softmax
    denominator in the same matmul.
"""

import sys
import os

sys.path.insert(0, "/opt/trn_rl_repo")

import numpy as np

B, S, C, X = 4, 2048, 16, 16
H = 8
CX = C * X  # 256
SURV = [0, 2, 3, 4, 8, 9, 10, 14]  # mv components surviving <q, ~k>
NSURV = len(SURV)  # 8
D = C * NSURV  # 128 packed q/k depth
SCALE = 1.0 / np.sqrt(32.0)
NCORES = 8
SB, JB, IB = 128, 512, 128  # s-tile, j-block, i-block sizes
NST, NJB, NIB = S // SB, S // JB, S // IB  # 16, 4, 16
NV = CX + 2  # 258: v columns + denominator ones column + even-pad (fp32r ISA)

_COMPILED = None  # (nc, name list) cache


def _head_weights(h, W_qkv, b_qkv, W_out):
    """Per-head block-diagonal weight construction (all float32)."""
    f32 = np.float32
    # row h*48 + c'*3 + p  (p: 0=q, 1=k, 2=v)
    Wh = W_qkv[h * 48 : (h + 1) * 48].reshape(C, 3, C)  # [c', p, c]
    bh = b_qkv[h * 48 : (h + 1) * 48].reshape(C, 3)  # [c', p]
    Wq, Wk, Wv = Wh[:, 0], Wh[:, 1], Wh[:, 2]  # each [c', c]
    qb, kb, vb = bh[:, 0], bh[:, 1], bh[:, 2]
    Wout_h = W_out[:, np.arange(C) * H + h]  # [o, c']
    Wvp = Wout_h @ Wv  # [o, c]
    vbp = Wout_h @ vb  # [o]

    # x_T row layout within half: r = (c - half*8)*16 + xi
    # packed q/k column layout: d = c'*8 + si  (si indexes SURV)
    Wq_bd = np.zeros((2, 128, 128), f32)
    Wk_bd = np.zeros((2, 128, 128), f32)
    Wvp_bd = np.zeros((2, 128, NV), f32)
    for half in range(2):
        for cl in range(8):
            c = half * 8 + cl
            for si, xs in enumerate(SURV):
                r = cl * 16 + xs
                Wq_bd[half, r, np.arange(C) * 8 + si] = SCALE * Wq[:, c]
                Wk_bd[half, r, np.arange(C) * 8 + si] = Wk[:, c]
            for xi in range(16):
                r = cl * 16 + xi
                Wvp_bd[half, r, np.arange(C) * 16 + xi] = Wvp[:, c]
    qb_col = np.zeros((128, 1), f32)
    kb_col = np.zeros((128, 1), f32)
    qb_col[np.arange(C) * 8, 0] = SCALE * qb  # si=0 <-> x component 0
    kb_col[np.arange(C) * 8, 0] = kb
    vbp_row = np.zeros((1, NV), f32)
    vbp_row[0, np.arange(C) * 16] = vbp  # xi'=0 scalar blade
    vbp_row[0, CX] = 1.0  # ones column -> softmax denominator
    return {
        "Wq_bd": Wq_bd,
        "Wk_bd": Wk_bd,
        "Wvp_bd": Wvp_bd,
        "qb_col": qb_col,
        "kb_col": kb_col,
        "vbp_row": vbp_row,
    }


def _build_program():
    import concourse.bass as bass
    import concourse.mybir as mybir
    import concourse.tile as tile
    from concourse import bacc
    from concourse.masks import make_identity

    f32 = mybir.dt.float32
    f32r = mybir.dt.float32r
    Exp = mybir.ActivationFunctionType.Exp

    nc = bacc.Bacc("TRN2", target_bir_lowering=False, debug=False)

    x_d = nc.dram_tensor("x", [B, S, CX], f32, kind="ExternalInput").ap()
    wq_d = nc.dram_tensor("Wq_bd", [2, 128, 128], f32, kind="ExternalInput").ap()
    wk_d = nc.dram_tensor("Wk_bd", [2, 128, 128], f32, kind="ExternalInput").ap()
    wvp_d = nc.dram_tensor("Wvp_bd", [2, 128, NV], f32, kind="ExternalInput").ap()
    qb_d = nc.dram_tensor("qb_col", [128, 1], f32, kind="ExternalInput").ap()
    kb_d = nc.dram_tensor("kb_col", [128, 1], f32, kind="ExternalInput").ap()
    vbp_d = nc.dram_tensor("vbp_row", [1, NV], f32, kind="ExternalInput").ap()
    y_d = nc.dram_tensor("y", [B, S, CX], f32, kind="ExternalOutput").ap()

    with tile.TileContext(nc) as tc:
        with (
            tc.tile_pool(name="const", bufs=1) as const,
            tc.tile_pool(name="xin", bufs=6) as xin,
            tc.tile_pool(name="xT", bufs=2) as xTp,
            tc.tile_pool(name="qk", bufs=2) as qkp,
            tc.tile_pool(name="vp", bufs=2) as vpp,
            tc.tile_pool(name="es", bufs=6) as esp,
            tc.tile_pool(name="yo", bufs=3) as yop,
            tc.tile_pool(name="psm", bufs=2, space="PSUM") as psm,
            tc.tile_pool(name="pss", bufs=2, space="PSUM") as pssp,
            tc.tile_pool(name="psy", bufs=1, space="PSUM") as psyp,
        ):
            ident = const.tile([128, 128], f32, tag="ident")
            make_identity(nc, ident[:])

            ones_s = const.tile([1, 512], f32, tag="ones_s")
            nc.vector.memset(ones_s[:], 1.0)
            ones_r = const.tile([1, 512], f32r, tag="ones_r")
            nc.vector.tensor_copy(out=ones_r[:], in_=ones_s[:])

            state = {}

            def load_consts():
                # emitted after phase 1 of b=0 so the first x DMA + PE
                # transposes aren't queued behind this prologue
                def load_w(name, dram, nvcols, eng):
                    st = const.tile([128, 2, nvcols], f32, tag=name + "_s", name=name + "_s")
                    for half in range(2):
                        eng.dma_start(out=st[:, half], in_=dram[half])
                    rt = const.tile([128, 2, nvcols], f32r, tag=name + "_r", name=name + "_r")
                    nc.vector.tensor_copy(out=rt[:], in_=st[:])
                    return rt

                state["wq"] = load_w("wq", wq_d, 128, nc.scalar)
                state["wk"] = load_w("wk", wk_d, 128, nc.gpsimd)
                state["wvp"] = load_w("wvp", wvp_d, NV, nc.scalar)

                qb_sb = const.tile([128, 1], f32, tag="qb", name="qb_sb")
                nc.gpsimd.dma_start(out=qb_sb[:], in_=qb_d[:])
                kb_sb = const.tile([128, 1], f32, tag="kb", name="kb_sb")
                nc.gpsimd.dma_start(out=kb_sb[:], in_=kb_d[:])
                vbp_s = const.tile([1, NV], f32, tag="vbp_s", name="vbp_s")
                nc.scalar.dma_start(out=vbp_s[:], in_=vbp_d[:])
                vbp_r = const.tile([1, NV], f32r, tag="vbp_r", name="vbp_r")
                nc.vector.tensor_copy(out=vbp_r[:], in_=vbp_s[:])
                # broadcast v-bias row to all 128 partitions once (ones ⊗ vbp)
                # (uses the yps0 bank, idle until b=0 attention)
                pvb = psyp.tile([128, NV], f32, tag="yps0", name="pvb")
                nc.tensor.matmul(pvb[:], ones_r[:, :128], vbp_r[:], start=True, stop=True)
                vbias_bc = const.tile([128, NV], f32, tag="vbias_bc", name="vbias_bc")
                nc.vector.tensor_copy(out=vbias_bc[:], in_=pvb[:])
                state["qb_sb"] = qb_sb
                state["kb_sb"] = kb_sb
                state["vbias_bc"] = vbias_bc

            try:
                n_rep = int(os.environ.get("BASS_REPEAT", "1"))
            except ValueError:
                n_rep = 1
            for rep in range(n_rep):
              for b in range(B):
                # ---- phase 1: load x[b], transpose to [(c,x), s] ----
                xT = [
                    xTp.tile([128, S], f32r, tag=f"xT{half}", name=f"xT{half}") for half in range(2)
                ]
                def load_quad(stq):
                    xt = xin.tile([128, 4, CX], f32, tag="x", name="xt")
                    src_ap = x_d[b, stq * 512 : (stq + 1) * 512, :].rearrange(
                        "(k p) c -> p k c", k=4, p=SB
                    )
                    if rep == 0 and b == 0 and stq == 0:
                        # split the very first load so transposes start sooner
                        for k in range(4):
                            nc.sync.dma_start(out=xt[:, k], in_=src_ap[:, k])
                    else:
                        nc.sync.dma_start(out=xt[:], in_=src_ap)
                    for half in range(2):
                        pt = psm.tile([128, 512], f32, tag="misc", name="pt")
                        for k in range(4):
                            nc.tensor.transpose(
                                pt[:, k * 128 : (k + 1) * 128],
                                xt[:, k, half * 128 : (half + 1) * 128],
                                ident[:],
                            )
                        nc.vector.tensor_copy(
                            out=xT[half][:, stq * 512 : (stq + 1) * 512], in_=pt[:, :512]
                        )

                load_quad(0)
                if rep == 0 and b == 0:
                    load_consts()
                wq, wk, wvp = state["wq"], state["wk"], state["wvp"]
                qb_sb, kb_sb = state["qb_sb"], state["kb_sb"]
                vbias_bc = state["vbias_bc"]

                # ---- phase 2: projections, interleaved per 512-col quad ----
                qp = qkp.tile([128, S], f32r, tag="qp")
                kp = qkp.tile([128, S], f32r, tag="kp")
                vp = vpp.tile([128, NST, NV], f32r, tag="vp")

                def proj_quad(sb4):
                    sl = slice(sb4 * JB, (sb4 + 1) * JB)
                    pq = psm.tile([128, 512], f32, tag="misc", name="pq")
                    nc.tensor.matmul(pq[:, :512], wq[:, 0], xT[0][:, sl], start=True, stop=False)
                    nc.tensor.matmul(pq[:, :512], wq[:, 1], xT[1][:, sl], start=False, stop=True)
                    nc.vector.tensor_scalar_add(out=qp[:, sl], in0=pq[:, :512], scalar1=qb_sb[:])
                    pk = psm.tile([128, 512], f32, tag="misc", name="pk")
                    nc.tensor.matmul(pk[:, :512], wk[:, 0], xT[0][:, sl], start=True, stop=False)
                    nc.tensor.matmul(pk[:, :512], wk[:, 1], xT[1][:, sl], start=False, stop=True)
                    nc.vector.tensor_scalar_add(out=kp[:, sl], in0=pk[:, :512], scalar1=kb_sb[:])
                    for st in range(sb4 * 4, sb4 * 4 + 4):
                        svl = slice(st * SB, (st + 1) * SB)
                        pv = psm.tile([128, 512], f32, tag="misc", name="pv")
                        nc.tensor.matmul(pv[:, :NV], xT[0][:, svl], wvp[:, 0], start=True, stop=False)
                        nc.tensor.matmul(pv[:, :NV], xT[1][:, svl], wvp[:, 1], start=False, stop=True)
                        nc.vector.tensor_add(out=vp[:, st], in0=pv[:, :NV], in1=vbias_bc[:])

                # ---- phase 3: attention (jb=0 interleaved with quad loads) ----
                LAG = 3

                def make_jb(jb):
                    jsl = slice(jb * JB, (jb + 1) * JB)
                    yps = [psyp.tile([128, NV], f32, tag=f"yps{js}", name=f"yps{js}") for js in range(4)]
                    es_q = {}

                    def produce(ib):
                        isl = slice(ib * IB, (ib + 1) * IB)
                        pss = pssp.tile([128, 512], f32, tag="ps_s", name="pss")
                        nc.tensor.matmul(pss[:], kp[:, isl], qp[:, jsl], start=True, stop=True)
                        es = esp.tile([128, 512], f32r, tag="es", name="es")
                        nc.scalar.activation(es[:], pss[:], Exp)
                        es_q[ib] = es

                    def consume(ib):
                        es = es_q.pop(ib)
                        for js in range(4):
                            nc.tensor.matmul(
                                yps[js][:],
                                es[:, js * 128 : (js + 1) * 128],
                                vp[:, ib],
                                start=(ib == 0),
                                stop=(ib == NIB - 1),
                            )

                    def finish():
                        ysb = yop.tile([128, 4, CX], f32, tag="ysb")
                        for js in range(4):
                            rec = yop.tile([128, 1], f32, tag="rec")
                            nc.vector.reciprocal(rec[:], yps[js][:, CX : CX + 1])
                            nc.vector.tensor_scalar_mul(
                                out=ysb[:, js], in0=yps[js][:, :CX], scalar1=rec[:]
                            )
                        dst_ap = y_d[b, jb * JB : (jb + 1) * JB, :].rearrange(
                            "(k p) c -> p k c", k=4, p=SB
                        )
                        nc.sync.dma_start(out=dst_ap, in_=ysb[:])

                    return produce, consume, finish

                # jb=0 pipelined against quads 1-3: produce(4q..4q+3) right
                # after quad q is projected, consume lags by LAG
                produce0, consume0, finish0 = make_jb(0)
                proj_quad(0)
                pr = cn = 0
                for q in range(NJB):
                    if q + 1 < NJB:
                        load_quad(q + 1)
                        proj_quad(q + 1)
                    while pr < 4 * (q + 1):
                        produce0(pr)
                        pr += 1
                        while cn < pr - LAG:
                            consume0(cn)
                            cn += 1
                while cn < NIB:
                    consume0(cn)
                    cn += 1
                finish0()

                for jb in range(1, NJB):
                    produce, consume, finish = make_jb(jb)
                    for ib in range(NIB + LAG):
                        if ib < NIB:
                            produce(ib)
                        if ib >= LAG:
                            consume(ib - LAG)
                    finish()

    nc.compile()
    return nc


def kernel(x, W_qkv, b_qkv, W_out, b_out):
    global _COMPILED
    from concourse import bass_utils

    x = np.ascontiguousarray(np.asarray(x, dtype=np.float32).reshape(B, S, CX))
    W_qkv = np.asarray(W_qkv, dtype=np.float32)
    b_qkv = np.asarray(b_qkv, dtype=np.float32)
    W_out = np.asarray(W_out, dtype=np.float32)
    b_out = np.asarray(b_out, dtype=np.float32)

    if _COMPILED is None:
        _COMPILED = _build_program()
    nc = _COMPILED

    in_maps = []
    for h in range(NCORES):
        w = _head_weights(h, W_qkv, b_qkv, W_out)
        in_maps.append({"x": x, **w})

    try:
        trace = bool(int(os.environ.get("BASS_PROFILE", "0")))
    except ValueError:
        trace = False
    try:
        res = bass_utils.run_bass_kernel_spmd(
            nc, in_maps, core_ids=list(range(NCORES)), trace=trace
        )
    except Exception:
        # transient NRT_EXEC_UNIT_UNRECOVERABLE observed on the tunneled
        # device; a fresh attempt recovers
        import time as _time

        _time.sleep(2.0)
        res = bass_utils.run_bass_kernel_spmd(
            nc, in_maps, core_ids=list(range(NCORES)), trace=trace
        )
    if trace:
        kernel.last_exec_time_ns = res.exec_time_ns
        kernel.last_results = res

    y = np.zeros((B, S, C, X), dtype=np.float64)
    for h in range(NCORES):
        y += res.results[h]["y"].astype(np.float64).reshape(B, S, C, X)
    y[:, :, :, 0] += b_out.astype(np.float64)[None, None, :]
    return y.astype(np.float32)

